# revision 23
# baseline (speedup 1.0000x reference)
"""GQA causal attention (B=1, S=2048, H=1024, 16 q-heads, 4 kv-heads, hd=64)
distributed over 8 TRN2 NeuronCores: tensor-parallel over query heads
(2 q-heads + their shared kv-head per core), x replicated. Per-core output
columns are concatenated on the host, which also performs the final softmax
divide (outputs ship as numerator rows + denominator row).

v17 design notes (85.5us baseline -> ~74.4us at this device's DVFS cap):
  - score matmuls pack the TWO HEADS of a key tile at PE row tiles 0/64,
    so pairs run concurrently; one partition-shift DMA per chunk.
  - causal trim exact at key-tile granularity; single 128-wide
    affine_select band per diagonal tile per head.
  - q/kv projections interleave per hid-chunk in two PSUM banks; chunk
    n+1's projection steps and v-transposes pace through chunk n's
    stream so the in-order PE queue stays dense.
  - PV(h0) lags scores by 3 key tiles; PV(h1) of chunk n-1 retained one
    chunk as always-ready PE filler. Pool depths carry slack (eq 24).
  - output [dim, query]: numerator + denominator rows, host divides.
  - steady state is ACT-bound at the DVFS cap (~92% scalar busy); the
    startup is DMA+clock-ramp bound (first exp ~24.5us); ~7.4us boot
    preamble and ~10us semaphore-reset epilogue are framework-fixed.

Optimization post-mortems (v18-v23, all reverted after on-device A/B):
  - K=64 row-group splitting of K=128 chains (proj/PV): only faster
    under throttle; ~7us SLOWER at the cap (74.4 vs 81.6+).
  - one persistent multi-bank PSUM tile as a manual ring: coarse
    whole-tile dep edges serialize the pipeline (74.9 -> 89.4us).
  - chunk-0 query-split (two 256-query sub-chunks to start the ACT
    spine ~6us earlier): crashes the device — two start/stop
    accumulation groups over disjoint column regions of one PSUM bank
    appear illegal, like mixing PE tile configs in one group.
  - HW constraints found: a PSUM accumulation group must keep one PE
    tile config; DVE reads at most one PSUM operand (NCC_IBVF027);
    gpsimd cannot access PSUM.
"""
from contextlib import ExitStack

import numpy as np
import ml_dtypes

import concourse.tile as tile
from concourse import bacc, mybir
from concourse.bass_utils import run_bass_kernel_spmd

F32 = mybir.dt.float32
BF16 = mybir.dt.bfloat16
S = 2048
NCORES = 8
SCALE = 1.0 / 32.0  # 1/sqrt(1024)
EXP = mybir.ActivationFunctionType.Exp
LAG = 3  # score->PV(h0) key-tile lag hiding the exp latency


def _make_identity(nc, ap, size):
    nc.gpsimd.memset(ap, 0.0)
    nc.gpsimd.affine_select(
        out=ap,
        in_=ap,
        compare_op=mybir.AluOpType.not_equal,
        fill=1.0,
        base=0,
        pattern=[[-1, size]],
        channel_multiplier=1,
    )


def _build_kernel(ctx: ExitStack, tc: "tile.TileContext", out, c0, xT13):
    nc = tc.nc

    const_pool = ctx.enter_context(tc.tile_pool(name="const", bufs=1))
    ident_bf = const_pool.tile([128, 128], BF16)
    warm = const_pool.tile([1, 1], F32)

    persist = ctx.enter_context(tc.tile_pool(name="persist", bufs=1))
    c0sb = persist.tile([128, 6144], BF16)  # [wq 8x128 | wkv 8x128 | x0 4096]
    qboth = persist.tile([128, S], BF16)    # h0 q at base 0, h1 q at base 64
    kshift = persist.tile([128, S], BF16)   # kT replicated at base 64
    v1 = persist.tile([128, 16, 65], BF16)  # [v | 1] tiles, [sk, hd+1]
    xs = [None] + [persist.tile([128, 4096], BF16, name=f"xn{n}") for n in (1, 2, 3)]
    kvns = [persist.tile([128, 512], BF16, name=f"kvn{n}") for n in range(4)]

    def wq_k(k):
        return c0sb[:, 128 * k:128 * (k + 1)]

    def wkv_k(k):
        return c0sb[:, 1024 + 128 * k:1024 + 128 * (k + 1)]

    def x_sl(n, k):
        if n == 0:
            return c0sb[:, 2048 + 512 * k:2048 + 512 * (k + 1)]
        return xs[n][:, 512 * k:512 * (k + 1)]

    nc.sync.dma_start(c0sb[:, 0:1024], c0[:, 0:1024])        # wq
    nc.sync.dma_start(c0sb[:, 2048:2560], c0[:, 2048:2560])  # x0 k0
    nc.sync.dma_start(c0sb[:, 1024:2048], c0[:, 1024:2048])  # wkv
    nc.sync.dma_start(c0sb[:, 2560:4096], c0[:, 2560:4096])  # x0 k1-3
    nc.sync.dma_start(c0sb[:, 4096:6144], c0[:, 4096:6144])  # x0 k4-7
    nc.sync.dma_start(xs[1][:, 0:2048], xT13[0, :, 0:2048])
    nc.sync.dma_start(xs[1][:, 2048:4096], xT13[0, :, 2048:4096])
    nc.sync.dma_start(xs[2][:, 0:2048], xT13[1, :, 0:2048])
    nc.sync.dma_start(xs[2][:, 2048:4096], xT13[1, :, 2048:4096])
    nc.sync.dma_start(xs[3][:], xT13[2])
    nc.scalar.memzero(warm[:])
    nc.scalar.activation(warm[:], warm[:], EXP)
    _make_identity(nc, ident_bf[:], 128)
    nc.vector.memset(v1[:, :, 64:65], 1.0)

    ppsum = ctx.enter_context(tc.tile_pool(name="proj_psum", bufs=1, space="PSUM"))
    scp = ctx.enter_context(tc.tile_pool(name="sc_psum", bufs=1, space="PSUM"))
    o2p = ctx.enter_context(tc.tile_pool(name="o2_psum", bufs=2, space="PSUM"))
    eqpool = ctx.enter_context(tc.tile_pool(name="eq", bufs=2))
    o2sbpool = ctx.enter_context(tc.tile_pool(name="o2sb", bufs=4))

    proj_psums = {}

    def proj_steps(n):
        steps = []

        def mk_mm(which, k):
            def emit():
                if which == "pq" and k == 0:
                    proj_psums[n] = {}
                if k == 0:
                    proj_psums[n][which] = ppsum.tile(
                        [128, 512], F32, tag=which, name=f"{which}{n}"
                    )
                p = proj_psums[n][which]
                w = wq_k(k) if which == "pq" else wkv_k(k)
                nc.tensor.matmul(
                    p[:], w, x_sl(n, k), start=(k == 0), stop=(k == 7)
                )
            return emit

        def cast_q():
            ns = slice(512 * n, 512 * (n + 1))
            nc.vector.tensor_copy(qboth[:, ns], proj_psums[n]["pq"][:])

        def cast_kv():
            ns = slice(512 * n, 512 * (n + 1))
            nc.vector.tensor_copy(kvns[n][:], proj_psums[n]["pkv"][:])
            nc.gpsimd.dma_start(kshift[64:128, ns], kvns[n][0:64, :])

        def mk_trv(t):
            def emit():
                trv = ppsum.tile([128, 64], BF16, tag="pq", name=f"trv{n}{t}")
                nc.tensor.transpose(
                    trv[:],
                    kvns[n][64:128, 128 * t:128 * (t + 1)],
                    ident_bf[64:128, 64:128],
                )
                nc.vector.tensor_copy(v1[:, 4 * n + t, 0:64], trv[:])
            return emit

        for k in range(8):
            steps.append(mk_mm("pq", k))
            steps.append(mk_mm("pkv", k))
        steps.append(cast_q)
        steps.append(cast_kv)
        trvs = [mk_trv(t) for t in range(4)]
        return steps, trvs

    eqs = {}
    o2s = {}

    def col_start(n, ki):
        return max(0, 128 * ki - 512 * n)

    def emit_scores(n, ki):
        s0 = col_start(n, ki)
        cols = slice(512 * n + s0, 512 * (n + 1))
        sq = scp.tile([128, 2, 512], F32, tag="sq", bufs=2, name=f"sq{n}_{ki}")
        lk = 128 * (ki % 4)
        nc.tensor.matmul(
            sq[:, 0, s0:512],
            kvns[ki // 4][0:64, lk:lk + 128],
            qboth[0:64, cols],
            start=True,
            stop=True,
        )
        nc.tensor.matmul(
            sq[:, 1, s0:512],
            kshift[64:128, 128 * ki:128 * (ki + 1)],
            qboth[64:128, cols],
            start=True,
            stop=True,
        )
        eq = eqpool.tile(
            [128, 2, 512], BF16, tag="eq", bufs=24, name=f"eq{n}_{ki}"
        )
        nc.scalar.activation(eq[:, :, s0:512], sq[:, :, s0:512], EXP, scale=SCALE)
        if ki >= 4 * n:
            for j in range(2):
                nc.gpsimd.affine_select(
                    out=eq[:, j, s0:s0 + 128],
                    in_=eq[:, j, s0:s0 + 128],
                    compare_op=mybir.AluOpType.is_ge,
                    fill=0.0,
                    base=0,
                    pattern=[[1, 128]],
                    channel_multiplier=-1,
                )
        eqs[(n, ki)] = eq

    def emit_pv(n, h, ki):
        nki = 4 * (n + 1)
        if ki == 0:
            o2s[(n, h)] = o2p.tile([65, 512], F32, tag="o2", name=f"o2_{n}{h}")
        o2 = o2s[(n, h)]
        s0 = col_start(n, ki)
        nc.tensor.matmul(
            o2[:, s0:512],
            v1[:, ki, :],
            eqs[(n, ki)][:, h, s0:512],
            start=(ki == 0),
            stop=(ki == nki - 1),
        )

    def emit_norm(n, h):
        o2sb = o2sbpool.tile([65, 512], BF16, tag="o2sb", name=f"o2sb{n}{h}")
        nc.vector.tensor_copy(o2sb[:], o2s[(n, h)][:])
        nc.sync.dma_start(out[n, h], o2sb[:])

    steps0, trvs0 = proj_steps(0)
    for st in steps0:
        st()

    for n in range(4):
        nki = 4 * (n + 1)
        if n + 1 < 4:
            psteps, ptrvs = proj_steps(n + 1)
            pending_proj = psteps + ptrvs
        else:
            pending_proj = []
        if n == 0:
            pending_proj = trvs0 + pending_proj
        h1jobs = list(range(4 * n)) if n > 0 else []
        h1_done = 0
        proj_done = 0
        for i in range(nki):
            target_h1 = min(len(h1jobs), (len(h1jobs) * (i + 1) * 4) // (3 * nki))
            while h1_done < target_h1:
                emit_pv(n - 1, 1, h1jobs[h1_done])
                h1_done += 1
            if n > 0 and h1_done == len(h1jobs) and h1_done > 0:
                emit_norm(n - 1, 1)
                h1_done += 1
            target_p = (len(pending_proj) * (i + 1)) // nki
            while proj_done < target_p:
                pending_proj[proj_done]()
                proj_done += 1
            emit_scores(n, i)
            if i >= LAG:
                emit_pv(n, 0, i - LAG)
        for ki in range(max(0, nki - LAG), nki):
            emit_pv(n, 0, ki)
        if n < 3:
            emit_norm(n, 0)
    for ki in range(4):
        emit_pv(3, 1, ki)
    emit_norm(3, 0)
    for ki in range(4, 16):
        emit_pv(3, 1, ki)
    emit_norm(3, 1)


def build_nc():
    nc = bacc.Bacc(
        "TRN2", target_bir_lowering=False, debug=False, num_devices=NCORES
    )
    c0 = nc.dram_tensor("c0", [128, 6144], BF16, kind="ExternalInput").ap()
    xT13 = nc.dram_tensor("xT13", [3, 128, 4096], BF16, kind="ExternalInput").ap()
    out = nc.dram_tensor("out", [4, 2, 65, 512], BF16, kind="ExternalOutput").ap()
    with tile.TileContext(nc) as tc, ExitStack() as ctx:
        _build_kernel(ctx, tc, out, c0, xT13)
    nc.compile()
    return nc


def make_in_maps(x, Wq, Wk, Wv):
    x = np.asarray(x, dtype=np.float32)
    Wq = np.asarray(Wq, dtype=np.float32)
    Wk = np.asarray(Wk, dtype=np.float32)
    Wv = np.asarray(Wv, dtype=np.float32)
    bf = ml_dtypes.bfloat16
    xh = np.ascontiguousarray(
        x[0].reshape(4, 512, 8, 128).transpose(0, 3, 2, 1).reshape(4, 128, 4096)
    ).astype(bf)
    xT13 = np.ascontiguousarray(xh[1:4])
    in_maps = []
    for d in range(NCORES):
        g = d // 2
        wq = (
            np.ascontiguousarray(
                Wq[128 * d:128 * (d + 1)].reshape(128, 8, 128).transpose(2, 1, 0)
            )
            .astype(bf)
            .reshape(128, 1024)
        )
        wkv = (
            np.ascontiguousarray(
                np.concatenate(
                    [Wk[64 * g:64 * (g + 1)], Wv[64 * g:64 * (g + 1)]], axis=0
                )
                .reshape(128, 8, 128)
                .transpose(2, 1, 0)
            )
            .astype(bf)
            .reshape(128, 1024)
        )
        c0 = np.concatenate([wq, wkv, xh[0]], axis=1)
        in_maps.append({"c0": np.ascontiguousarray(c0), "xT13": xT13})
    return in_maps


_NC_CACHE = None


def _get_nc():
    global _NC_CACHE
    if _NC_CACHE is None:
        _NC_CACHE = build_nc()
    return _NC_CACHE


def _run_once(in_maps):
    res = run_bass_kernel_spmd(_get_nc(), in_maps, core_ids=list(range(NCORES)))
    outs = []
    ok = True
    for d in range(NCORES):
        o = np.asarray(res.results[d]["out"]).astype(np.float32)  # [4,2,65,512]
        den = o[:, :, 64:65, :]
        # softmax denominators are sums of exp(~0) terms: positive, O(1)..O(2048).
        # a transient bad execute shows up as nonfinite values or junk denoms.
        if not (np.isfinite(o).all() and (den > 1e-2).all() and (den < 1e7).all()):
            ok = False
        y = o[:, :, 0:64, :] / den
        outs.append(y.transpose(0, 3, 1, 2).reshape(S, 128))  # [2048, 128]
    return np.concatenate(outs, axis=1)[None, :, :], ok


def kernel(x, Wq, Wk, Wv):
    in_maps = make_in_maps(x, Wq, Wk, Wv)
    full, ok = _run_once(in_maps)
    if not ok:  # transient device hiccup: retry once
        full, _ = _run_once(in_maps)
    return full
